# revision 11
# baseline (speedup 1.0000x reference)
"""3-layer GAT on 8 trn2 NeuronCores.

Strategy (v5, all-fp16)
-----------------------
Nodes are permuted (per-core in-degree sort, 2x strict-lo refinement) and
dealt to 8 cores; each core owns 6272 contiguous ids (6250 real + 22 pad).
One Bass program runs 3 times (one launch per GAT layer); the host
concatenates per-core outputs between launches.

Each core sees a ROTATED node numbering (its own nodes first), so its dst
rows sit at table rows [0, 6272) at compile time.  Per launch, each core:

  1. Builds the h table T [50176 rows x 256B] in DRAM: BN-affine+relu on
     XT, then stride-8 matmuls per 1024-node chunk so each SBUF partition
     holds 8 consecutive table rows -> 2KB-contiguous DMA writes.  Row =
     pure h fp16 x128 (no attention coefficients stored).  The designated
     ELL pad rows get h poisoned to -1e3*sign(a_src) so recomputed pad
     logits are hugely negative -> exp == 0.
  2. Processes dst blocks (128 dsts): the self loop is slot 0 of each
     lane's lo list, so there is no separate self path.  3 dma_gathers
     per group (lo split in two + hi) round-robined over 4 SWDGE queues.
     Attention logits are recomputed on-device from the gathered h:
     al_s = reduce(h * a_src), al_d from the self slot; prelu+exp on ACT
     (exp accumulates the softmax denominator per head), alpha in fp16,
     weighted rows reduced with a transposed PE accumulate chain
     (lhsT=weighted chunk, rhs=identity) -> PSUM [ch, dst] directly.
  3. Head-mix matmul; bias + BN partial sums via fused DVE reduces.
"""
import os
import numpy as np

import concourse.bass as bass
import concourse.bacc as bacc
import concourse.mybir as mybir
import concourse.tile as tile
from concourse import bass_utils
from concourse.masks import make_identity
from concourse.tile_sem_assignment import PROC_NAME_TO_IDX

_IDX_TO_PROC = {v: k for k, v in PROC_NAME_TO_IDX.items()}

def _bc(ap, pos, count):
    """Insert a step-0 (broadcast) axis into an AP at position pos."""
    lst = [list(x) for x in ap.ap]
    lst.insert(pos, [0, count])
    return bass.AP(ap.tensor, ap.offset, lst)


F32 = mybir.dt.float32
BF16 = mybir.dt.bfloat16
F16 = mybir.dt.float16
I16 = mybir.dt.int16

N = 50000
E = 800000
H = 2
CH = 64
IN = 128
OUT = 64
EPS = 1e-5
SLOPE = 0.2
POISON = -1e3            # h poison magnitude for pad rows

N_CORES = 8
PER_CORE = 6272            # 49 * 128
NPAD = N_CORES * PER_CORE  # 50176
NBLK = PER_CORE // 128     # 49
REAL_PER_CORE = N // N_CORES  # 6250
D = 128                    # h channels = table row elems (bf16) = 256B
LO_END = 32768             # lo window [0, LO_END)
HI_START = NPAD - 32768    # hi window [HI_START, NPAD)
NQ = 4
GRP = 1                    # dst blocks per gather group
PADROW_LO = REAL_PER_CORE              # 6250 (own first pad row, poisoned)
PADROW_HI = 7 * PER_CORE + REAL_PER_CORE  # 50154 (also poisoned)
NGRP = (NBLK + GRP - 1) // GRP


# ----------------------------------------------------------------- host prep

def _wrap_idxs(flat):
    """flat [n] int -> dma_gather idx layout [128, n/16] int16 (wrapped in 16
    partitions, i = s*16 + p, replicated across the 8 q7 core groups)."""
    n = flat.shape[0]
    w = flat.reshape(n // 16, 16).T.astype(np.int16)
    return np.tile(w, (8, 1))


def preprocess(edge_index):
    """Build node permutation and per-core ELL grids (core-rotated ids).

    The self loop is prepended to each lane's lo list (its rotated id is
    always < PER_CORE, i.e. in the lo window)."""
    src = edge_index[0].astype(np.int64)
    dst = edge_index[1].astype(np.int64)

    indeg = np.bincount(dst, minlength=N)  # slots/dst (self loop NOT counted)
    order = np.argsort(-indeg, kind="stable")
    core_of = np.empty(N, np.int32)
    for i in range(N):
        r = i % (2 * N_CORES)
        core_of[order[i]] = r if r < N_CORES else 2 * N_CORES - 1 - r

    def numbering(skey):
        new_id = np.empty(N, np.int64)
        for c in range(N_CORES):
            nodes = np.where(core_of == c)[0]
            if skey is None:
                kk = np.lexsort((nodes, -indeg[nodes]))
            else:
                kk = np.lexsort((nodes, skey[nodes], -indeg[nodes]))
            new_id[nodes[kk]] = c * PER_CORE + np.arange(len(nodes))
        return new_id

    nid = numbering(None)
    for _ in range(2):
        rel = (nid[src] - core_of[dst].astype(np.int64) * PER_CORE) % NPAD
        sl = np.bincount(dst, weights=(rel < HI_START).astype(np.float64),
                         minlength=N)
        nid = numbering(sl)
    new_id = nid

    ns = new_id[src]
    nd = new_id[dst]
    o = np.argsort(nd, kind="stable")
    ns, nd = ns[o], nd[o]
    starts = np.searchsorted(nd, np.arange(NPAD))
    ends = np.searchsorted(nd, np.arange(NPAD) + 1)

    # per-group window budgeting on ROTATED source positions.  Shared KLO is
    # fixed first (max over cores of must-lo), THEN KHI is sized against the
    # shared KLO -- the larger lo budget absorbs more overflow everywhere.
    KLO = np.zeros(NGRP, np.int64)
    KHI = np.zeros(NGRP, np.int64)
    klo_min_a = np.zeros((N_CORES, NGRP), np.int64)
    khi_min_a = np.zeros((N_CORES, NGRP), np.int64)
    deg_max_a = np.zeros((N_CORES, NGRP), np.int64)
    lists = {}
    for c in range(N_CORES):
        base_c = c * PER_CORE
        for gi in range(NGRP):
            b0 = gi * GRP
            gsz = min(GRP, NBLK - b0)
            klo_min = khi_min = deg_max = 0
            for j in range(gsz):
                for p in range(128):
                    d_local = (b0 + j) * 128 + p
                    d_node = base_c + d_local
                    sl_ = ns[starts[d_node]:ends[d_node]]
                    rel = (sl_ - base_c) % NPAD
                    must_lo = rel[rel < HI_START]
                    must_hi = rel[rel >= LO_END]
                    mid = rel[(rel >= HI_START) & (rel < LO_END)]
                    # self loop: always lo (d_local < PER_CORE < HI_START).
                    # Pad lanes get the poisoned PADROW instead -- their own
                    # rows are nonzero once BN bias kicks in (layers 1+), and
                    # a live self-slot would emit junk into pad columns and
                    # corrupt the BN partial sums.
                    selfrow = d_local if d_local < REAL_PER_CORE else PADROW_LO
                    must_lo = np.concatenate([[selfrow], must_lo])
                    lists[(c, b0 + j, p)] = (must_lo, must_hi, mid)
                    klo_min = max(klo_min, len(must_lo))
                    khi_min = max(khi_min, len(must_hi))
                    deg_max = max(deg_max, len(rel) + 1)
            klo_min_a[c, gi] = klo_min
            khi_min_a[c, gi] = khi_min
            deg_max_a[c, gi] = deg_max
    KLO = klo_min_a.max(axis=0)
    KHI = np.maximum(khi_min_a, deg_max_a - KLO[None, :]).max(axis=0)

    GSZ = [min(GRP, NBLK - gi * GRP) for gi in range(NGRP)]
    tot_slots = int(sum((KLO[g] + KHI[g]) * GSZ[g] for g in range(NGRP)) * 128)
    tot_edges = len(ns) // N_CORES + PER_CORE
    print(f"[prep] slots/core {tot_slots} vs edges+self/core ~{tot_edges} "
          f"(pad {tot_slots / tot_edges - 1:.1%})")

    # per-group grids, block-major sub-ranges inside each window region
    SLO = int(sum(KLO[g] * GSZ[g] for g in range(NGRP)))
    SHI = int(sum(KHI[g] * GSZ[g] for g in range(NGRP)))
    olo = np.concatenate([[0], np.cumsum([KLO[g] * GSZ[g] for g in range(NGRP)])]).astype(int)
    ohi = np.concatenate([[0], np.cumsum([KHI[g] * GSZ[g] for g in range(NGRP)])]).astype(int)

    glo_w = np.zeros((N_CORES, 128, 8 * SLO), np.int16)
    ghi_w = np.zeros((N_CORES, 128, 8 * SHI), np.int16)
    for c in range(N_CORES):
        for gi in range(NGRP):
            b0 = gi * GRP
            gsz = GSZ[gi]
            klo, khi = int(KLO[gi]), int(KHI[gi])
            glo_grp = np.zeros((128, gsz * klo), np.int64)
            ghi_grp = np.zeros((128, gsz * khi), np.int64)
            for j in range(gsz):
                for p in range(128):
                    must_lo, must_hi, mid = lists[(c, b0 + j, p)]
                    lo = list(must_lo)
                    hi = list(must_hi)
                    mid = list(mid)
                    room_lo = klo - len(lo)
                    lo += mid[:room_lo]
                    hi += mid[room_lo:]
                    assert len(lo) <= klo and len(hi) <= khi
                    glo_grp[p, j * klo:j * klo + len(lo)] = lo
                    glo_grp[p, j * klo + len(lo):(j + 1) * klo] = PADROW_LO
                    ghi_grp[p, j * khi:j * khi + len(hi)] = hi
                    ghi_grp[p, j * khi + len(hi):(j + 1) * khi] = PADROW_HI
            nlo = gsz * klo
            klo2 = nlo // 2
            for j0, j1 in ((0, klo2), (klo2, nlo)):
                if j1 > j0:
                    fl = glo_grp[:, j0:j1].T.reshape(-1)
                    glo_w[c, :, 8 * (olo[gi] + j0):8 * (olo[gi] + j1)] = _wrap_idxs(fl)
            if khi > 0:
                fh = ghi_grp.T.reshape(-1) - HI_START
                ghi_w[c, :, 8 * ohi[gi]:8 * ohi[gi + 1]] = _wrap_idxs(fh)

    return dict(new_id=new_id, KLO=KLO.tolist(), KHI=KHI.tolist(), GSZ=GSZ,
                glo=glo_w, ghi=ghi_w)


# ----------------------------------------------------------------- builder

def build(KLO, KHI, GSZ):
    nc = bacc.Bacc(None, target_bir_lowering=False, debug=False,
                   num_devices=N_CORES, num_swdge_queues=NQ)
    SLO = int(sum(KLO[g] * GSZ[g] for g in range(NGRP)))
    SHI = int(sum(KHI[g] * GSZ[g] for g in range(NGRP)))

    xt = nc.dram_tensor("xt", [128, NPAD], F16, kind="ExternalInput")
    part = nc.dram_tensor("part", [128, 16], F32, kind="ExternalInput")
    gvec = nc.dram_tensor("gvec", [128, 1], F32, kind="ExternalInput")
    bevec = nc.dram_tensor("bevec", [128, 1], F32, kind="ExternalInput")
    srel = nc.dram_tensor("srel", [128, 1], F32, kind="ExternalInput")
    wtmat = nc.dram_tensor("wtmat", [128, 128], F32, kind="ExternalInput")
    mmat = nc.dram_tensor("mmat", [128, 128], F16, kind="ExternalInput")
    biasv = nc.dram_tensor("biasv", [128, 1], F32, kind="ExternalInput")
    asrcT = nc.dram_tensor("asrcT", [128, 128], F16, kind="ExternalInput")
    adstT = nc.dram_tensor("adstT", [128, 128], F16, kind="ExternalInput")
    poisr = nc.dram_tensor("poisr", [128, 128], F16, kind="ExternalInput")
    glod = nc.dram_tensor("glo", [128, 8 * SLO], I16, kind="ExternalInput")
    ghid = nc.dram_tensor("ghi", [128, 8 * SHI], I16, kind="ExternalInput")

    outb = nc.dram_tensor("outb", [128, PER_CORE], F32, kind="ExternalOutput")
    parts = nc.dram_tensor("parts", [128, 2], F32, kind="ExternalOutput")

    tbl = nc.dram_tensor("tbl", [LO_END, D], F16)      # lo window rows
    tblh = nc.dram_tensor("tblh", [NPAD - HI_START, D], F16)  # hi window

    with tile.TileContext(nc) as tc:
        with (
            tc.tile_pool(name="const", bufs=1) as cpool,
            tc.tile_pool(name="norm", bufs=3) as npool,
            tc.tile_pool(name="tw", bufs=2) as twpool,
            tc.tile_pool(name="g", bufs=5) as gpool,
            tc.tile_pool(name="gw", bufs=9) as gwpool,
            tc.tile_pool(name="work", bufs=3) as wpool,
            tc.tile_pool(name="work2", bufs=3) as w2pool,
            tc.tile_pool(name="small", bufs=6) as spool,
            tc.tile_pool(name="acc", bufs=1) as apool,
            tc.tile_pool(name="ps", bufs=2, space="PSUM") as pspool,
            tc.tile_pool(name="ps2", bufs=3, space="PSUM") as ps2pool,
            tc.tile_pool(name="ps3", bufs=2, space="PSUM") as ps3pool,
        ):
            ident = cpool.tile([128, 128], F32, tag="ident")
            make_identity(nc, ident[:])
            identB = cpool.tile([128, 128], F16, tag="identB")
            nc.vector.tensor_copy(identB[:], ident[:])

            # --- BN params ------------------------------------------------
            pt = cpool.tile([128, 16], F32, tag="pt")
            nc.sync.dma_start(pt[:], part.ap())
            gv = cpool.tile([128, 1], F32, tag="gv")
            nc.sync.dma_start(gv[:], gvec.ap())
            bev = cpool.tile([128, 1], F32, tag="bev")
            nc.sync.dma_start(bev[:], bevec.ap())
            sv = cpool.tile([128, 1], F32, tag="sv")
            nc.sync.dma_start(sv[:], srel.ap())

            s1 = cpool.tile([128, 1], F32, tag="s1")
            s2 = cpool.tile([128, 1], F32, tag="s2")
            nc.vector.reduce_sum(s1[:], pt[:, 0:8], axis=mybir.AxisListType.X)
            nc.vector.reduce_sum(s2[:], pt[:, 8:16], axis=mybir.AxisListType.X)
            mu = cpool.tile([128, 1], F32, tag="mu")
            nc.vector.tensor_scalar_mul(mu[:], s1[:], 1.0 / N)
            msq = cpool.tile([128, 1], F32, tag="msq")
            nc.vector.tensor_scalar_mul(msq[:], s2[:], 1.0 / N)
            var = cpool.tile([128, 1], F32, tag="var")
            nc.vector.tensor_tensor(out=var[:], in0=mu[:], in1=mu[:],
                                    op=mybir.AluOpType.mult)
            nc.vector.tensor_tensor(out=var[:], in0=msq[:], in1=var[:],
                                    op=mybir.AluOpType.subtract)
            sd = cpool.tile([128, 1], F32, tag="sd")
            epsT = cpool.tile([128, 1], F32, tag="epsT")
            nc.vector.memset(epsT[:], EPS)
            nc.scalar.activation(sd[:], var[:], mybir.ActivationFunctionType.Sqrt,
                                 bias=epsT[:], scale=1.0)
            ra = cpool.tile([128, 1], F32, tag="ra")
            nc.vector.reciprocal(ra[:], sd[:])
            av = cpool.tile([128, 1], F32, tag="av")
            nc.vector.tensor_tensor(out=av[:], in0=ra[:], in1=gv[:],
                                    op=mybir.AluOpType.mult)
            bv = cpool.tile([128, 1], F32, tag="bv")
            nc.vector.tensor_tensor(out=bv[:], in0=mu[:], in1=av[:],
                                    op=mybir.AluOpType.mult)
            nc.vector.tensor_tensor(out=bv[:], in0=bev[:], in1=bv[:],
                                    op=mybir.AluOpType.subtract)

            wtt = cpool.tile([128, 128], F32, tag="wtt")
            nc.sync.dma_start(wtt[:], wtmat.ap())
            web = cpool.tile([128, 128], F16, tag="web")
            nc.vector.tensor_copy(web[:], wtt[:])
            mm = cpool.tile([128, 128], F16, tag="mm")
            nc.sync.dma_start(mm[:], mmat.ap())
            bi = cpool.tile([128, 1], F32, tag="bi")
            nc.sync.dma_start(bi[:], biasv.ap())
            asr = cpool.tile([128, 128], F16, tag="asr")
            nc.sync.dma_start(asr[:], asrcT.ap())
            ads = cpool.tile([128, 128], F16, tag="ads")
            nc.sync.dma_start(ads[:], adstT.ap())
            poi = cpool.tile([128, 128], F16, tag="poi")
            nc.sync.dma_start(poi[:], poisr.ap())
            slp = cpool.tile([128, 1], F32, tag="slp")
            nc.vector.memset(slp[:], SLOPE)
            glall = cpool.tile([128, 8 * SLO], I16, tag="glall")
            nc.sync.dma_start(glall[:], glod.ap())
            ghall = cpool.tile([128, 8 * SHI], I16, tag="ghall")
            nc.scalar.dma_start(ghall[:], ghid.ap())

            # --- table build: T[r] = relu_s(bn(x))^T @ W -------------------
            # stride-8 lhsT: psum sub-chunk i holds nodes {r0 + i + 8p};
            # partition p accumulates 8 consecutive rows -> 2KB contiguous
            # DMA per partition.  Two 4-matmul psum rounds per 1024 nodes.
            CH_N = 1024

            def build_chunk(r0):
                xn = npool.tile([128, CH_N], F16, tag="xn")
                eng_x = nc.sync if (r0 // CH_N) % 2 == 0 else nc.scalar
                eng_x.dma_start(xn[:], xt.ap()[:, r0:r0 + CH_N])
                u = npool.tile([128, CH_N], F16, tag="u")
                # prelu(alpha=S): S=1 -> identity, S=0 -> relu
                nc.scalar.activation(u[:], xn[:],
                                     mybir.ActivationFunctionType.Prelu,
                                     bias=bv[:], scale=av[:], alpha=sv[:])
                hb = twpool.tile([128, 8 * D], F16, tag="hb")
                u_ap = u[:]
                for half in range(2):
                    hp = pspool.tile([128, 4 * D], F32, tag="hp", space="PSUM")
                    for i4 in range(4):
                        i = half * 4 + i4
                        lhsT = bass.AP(u.tensor, u_ap.offset + i,
                                       [list(u_ap.ap[0]), [8, 128]])
                        nc.tensor.matmul(hp[:, i4 * D:(i4 + 1) * D],
                                         lhsT=lhsT, rhs=web[:],
                                         start=True, stop=True)
                    eng_c = nc.vector if half == 0 else nc.scalar
                    if half == 0:
                        nc.vector.tensor_copy(
                            hb[:, half * 4 * D:(half + 1) * 4 * D], hp[:])
                    else:
                        nc.scalar.copy(
                            hb[:, half * 4 * D:(half + 1) * 4 * D], hp[:])
                if r0 < LO_END:
                    nc.scalar.dma_start(
                        bass.AP(tbl, r0 * D, [[8 * D, 128], [1, 8 * D]]),
                        hb[:])
                if r0 >= HI_START:
                    nc.sync.dma_start(
                        bass.AP(tblh, (r0 - HI_START) * D,
                                [[8 * D, 128], [1, 8 * D]]),
                        hb[:])
                if r0 + CH_N == LO_END:
                    # poison lo-window pad row (gather target for lo fill)
                    nc.scalar.dma_start(
                        bass.AP(tbl, PADROW_LO * D, [[D, 1], [1, D]]),
                        poi[0:1, :])

            # --- per-group aggregation ------------------------------------
            pactl = apool.tile([128, 2 * NBLK], F32, tag="pactl")
            olo = np.concatenate(
                [[0], np.cumsum([KLO[g] * GSZ[g] for g in range(NGRP)])]).astype(int)
            ohi = np.concatenate(
                [[0], np.cumsum([KHI[g] * GSZ[g] for g in range(NGRP)])]).astype(int)
            NWARM = 8
            # warm prefix = smallest blocks (fit 9 g-tiles in SBUF);
            # then biggest-first so the LAST computed block is small
            # (short engine tail after the final gather)
            order = (list(range(NGRP - 1, NGRP - 1 - NWARM, -1))
                     + list(range(0, NGRP - NWARM)))
            pend = []

            def do_lo(gi, pos):
                gsz = GSZ[gi]
                klo, khi = KLO[gi], KHI[gi]
                k = klo + khi
                nlo = gsz * klo
                pool = gwpool if pos < NWARM else gpool
                g = pool.tile([128, gsz * k * D], F16,
                              tag="gw" if pos < NWARM else "g")
                g3 = g[:].rearrange("p (k d) -> p k d", d=D)
                nc.gpsimd.dma_gather(
                    out_ap=g3[:, 0:nlo, :], in_ap=tbl.ap(),
                    idxs_ap=glall[:, 8 * olo[gi]:8 * olo[gi + 1]],
                    num_idxs=128 * nlo, num_idxs_reg=128 * nlo,
                    elem_size=D, single_packet=False, queue_num=0)
                return (gi, g, g3)

            def do_hi_and_compute(st):
                gi, g, g3 = st
                b0 = gi * GRP
                gsz = GSZ[gi]
                klo, khi = KLO[gi], KHI[gi]
                k = klo + khi
                SE = k * H
                SW = k * D
                nlo = gsz * klo
                nhi = gsz * khi
                if khi > 0:
                    nc.gpsimd.dma_gather(
                        out_ap=g3[:, nlo:nlo + nhi, :],
                        in_ap=tblh.ap(),
                        idxs_ap=ghall[:, 8 * ohi[gi]:8 * ohi[gi + 1]],
                        num_idxs=128 * nhi, num_idxs_reg=128 * nhi,
                        elem_size=D, single_packet=False, queue_num=0)

                pstr_g = list(g[:].ap[0])
                g_off = g[:].offset

                # al_s for every slot: reduce(h * a_src) over each head's
                # 64 channels.  m layout = g layout; alv [slot, head].
                m = wpool.tile([128, gsz * k * D], F16, tag="m")
                nc.vector.tensor_tensor(
                    out=m[:], in0=g[:],
                    in1=_bc(asr[:], 1, gsz * k),
                    op=mybir.AluOpType.mult)
                alv = spool.tile([128, gsz * SE], F32, tag="alv")
                pstr_e = list(alv[:].ap[0])
                alv_off = alv[:].offset
                nc.vector.reduce_sum(
                    bass.AP(alv.tensor, alv_off, [pstr_e, [H, gsz * k], [1, H]]),
                    bass.AP(m.tensor, m[:].offset,
                            [list(m[:].ap[0]), [D, gsz * k], [CH, H], [1, CH]]),
                    axis=mybir.AxisListType.X)

                # al_d from the self slot (slot 0 of each block's lo range)
                md = spool.tile([128, gsz * D], F16, tag="md")
                selfg = bass.AP(g.tensor, g_off,
                                [pstr_g, [klo * D, gsz], [1, D]])
                nc.vector.tensor_tensor(
                    out=md[:].rearrange("p (g d) -> p g d", d=D),
                    in0=selfg, in1=_bc(ads[:], 1, gsz),
                    op=mybir.AluOpType.mult)
                ald = spool.tile([128, gsz * H], F32, tag="ald")
                nc.vector.reduce_sum(
                    bass.AP(ald.tensor, ald[:].offset,
                            [list(ald[:].ap[0]), [H, gsz], [1, H]]),
                    bass.AP(md.tensor, md[:].offset,
                            [list(md[:].ap[0]), [D, gsz], [CH, H], [1, CH]]),
                    axis=mybir.AxisListType.X)

                # per head: ee = prelu(alv + al_d); ex = exp(ee) with the
                # softmax denominator accumulated per partition
                ex = spool.tile([128, gsz * SE], F32, tag="ex")
                ex_off = ex[:].offset
                den = spool.tile([128, gsz * H], F32, tag="den")
                for j in range(gsz):
                    for hh in range(H):
                        eap = bass.AP(ex.tensor, ex_off + j * SE + hh,
                                      [pstr_e, [H, k]])
                        aap = bass.AP(alv.tensor, alv_off + j * SE + hh,
                                      [pstr_e, [H, k]])
                        nc.scalar.activation(
                            eap, aap, mybir.ActivationFunctionType.Prelu,
                            bias=ald[:, j * H + hh:j * H + hh + 1],
                            alpha=slp[:])
                        nc.scalar.activation(
                            eap, eap, mybir.ActivationFunctionType.Exp,
                            accum_out=den[:, j * H + hh:j * H + hh + 1])
                nc.vector.tensor_scalar_add(den[:], den[:], 1e-16)
                rr_ = spool.tile([128, gsz * H], F32, tag="rr")
                nc.vector.reciprocal(rr_[:], den[:])

                # alpha = ex * (1/den)
                exr = spool.tile([128, gsz * SE], F16, tag="exr")
                exr_off = exr[:].offset
                nc.vector.tensor_tensor(
                    out=bass.AP(exr.tensor, exr_off,
                                [pstr_e, [SE, gsz], [H, k], [1, H]]),
                    in0=bass.AP(ex.tensor, ex_off,
                                [pstr_e, [SE, gsz], [H, k], [1, H]]),
                    in1=_bc(bass.AP(rr_.tensor, rr_[:].offset,
                                    [list(rr_[:].ap[0]), [H, gsz], [1, H]]), 2, k),
                    op=mybir.AluOpType.mult)

                # weighted rows (fp16), separate tile so the g buffer
                # frees as soon as the DVE multiplies finish
                wb = w2pool.tile([128, gsz * k * D], F16, tag="wb")
                wb_off = wb[:].offset
                pstr_w = list(wb[:].ap[0])
                for hh in range(H):
                    nc.vector.tensor_tensor(
                        out=bass.AP(wb.tensor, wb_off + hh * CH,
                                    [pstr_w, [SW, gsz], [D, k], [1, CH]]),
                        in0=bass.AP(g.tensor, g_off + hh * CH,
                                    [pstr_g, [SW, gsz], [D, k], [1, CH]]),
                        in1=bass.AP(exr.tensor, exr_off + hh,
                                    [pstr_e, [SE, gsz], [H, k], [0, CH]]),
                        op=mybir.AluOpType.mult)

                for j in range(gsz):
                    b = b0 + j
                    P1 = ps2pool.tile([128, 128], F32, tag="P1", space="PSUM")
                    for i in range(k):
                        nc.tensor.matmul(
                            P1[:], lhsT=wb[:, j * SW + i * D:j * SW + (i + 1) * D],
                            rhs=identB[:], start=(i == 0), stop=(i == k - 1))

                    uts = spool.tile([128, 128], F16, tag="uts")
                    nc.scalar.copy(uts[:], P1[:])
                    otp = ps3pool.tile([128, 128], F32, tag="otp", space="PSUM")
                    nc.tensor.matmul(otp[:], lhsT=mm[:], rhs=uts[:],
                                     start=True, stop=True)
                    # ots = otp + bias, with BN partial sums accumulated on
                    # the ACT engine (Prelu alpha=1 == identity)
                    ots = spool.tile([128, 128], F32, tag="ots")
                    nc.scalar.activation(ots[:], otp[:],
                                         mybir.ActivationFunctionType.Prelu,
                                         bias=bi[:], alpha=1.0,
                                         accum_out=pactl[:, 2 * b:2 * b + 1])
                    sq = spool.tile([128, 128], F32, tag="sq")
                    nc.scalar.activation(sq[:], ots[:],
                                         mybir.ActivationFunctionType.Square,
                                         accum_out=pactl[:, 2 * b + 1:2 * b + 2])

                    nc.sync.dma_start(outb.ap()[:, b * 128:(b + 1) * 128], ots[:])

            # part A: build the lo window (rows [0, LO_END)), then start
            # the warm groups' lo gathers while part B builds the hi window
            for r0 in range(0, LO_END, CH_N):
                build_chunk(r0)
            for pos in range(NWARM):
                pend.append(do_lo(order[pos], pos))
            for r0 in range(LO_END, NPAD, CH_N):
                build_chunk(r0)
            # poison hi-window pad row
            nc.sync.dma_start(
                bass.AP(tblh, (PADROW_HI - HI_START) * D, [[D, 1], [1, D]]),
                poi[0:1, :])
            for stw in pend:
                do_hi_and_compute(stw)
            for pos in range(NWARM, NGRP):
                st = do_lo(order[pos], pos)
                do_hi_and_compute(st)

            pacc = apool.tile([128, 2], F32, tag="pacc")
            nc.vector.reduce_sum(
                bass.AP(pacc.tensor, pacc[:].offset,
                        [list(pacc[:].ap[0]), [1, 2], [1, 1]]),
                bass.AP(pactl.tensor, pactl[:].offset,
                        [list(pactl[:].ap[0]), [1, 2], [2, NBLK]]),
                axis=mybir.AxisListType.X)
            nc.sync.dma_start(parts.ap(), pacc[:])

    # align each gather's SWDGE queue with its Tile-assigned DMASW sem lane
    for bb in nc.main_func.blocks:
        for ins in bb.instructions:
            if isinstance(ins, mybir.InstDMAGatherAnt):
                nm = _IDX_TO_PROC.get(ins.bass_scheduled_proc, "")
                if nm.startswith("DMASW"):
                    ins.queue_num = int(nm[5:]) % NQ

    nc.compile()
    return nc


# ----------------------------------------------------------------- driver

_TRACE = bool(os.environ.get("KERNEL_TRACE"))
LAST_EXEC_NS = []
DBG_LAYERS = []


def kernel(x, edge_index, W0, a_src0, a_dst0, b0, g0, be0,
           W1, a_src1, a_dst1, b1, g1, be1,
           W2, a_src2, a_dst2, b2):
    global LAST_EXEC_NS
    LAST_EXEC_NS = []

    prep = preprocess(np.asarray(edge_index))
    new_id = prep["new_id"]

    nc = build(prep["KLO"], prep["KHI"], prep["GSZ"])

    xp = np.zeros((NPAD, IN), np.float32)
    xp[new_id] = np.asarray(x, np.float32)

    eye = np.eye(128, dtype=np.float32)
    mix2 = np.zeros((128, 128), np.float32)
    mix2[0:64, 0:64] = 0.5 * np.eye(64)
    mix2[64:128, 0:64] = 0.5 * np.eye(64)

    layers = [
        dict(W=W0, a_src=a_src0, a_dst=a_dst0, bias=np.asarray(b0),
             g=np.full(128, np.sqrt(EPS), np.float32), be=np.zeros(128, np.float32),
             s=1.0, mix=eye),
        dict(W=W1, a_src=a_src1, a_dst=a_dst1, bias=np.asarray(b1),
             g=np.asarray(g0), be=np.asarray(be0), s=0.0, mix=eye),
        dict(W=W2, a_src=a_src2, a_dst=a_dst2, bias=np.concatenate(
            [np.asarray(b2), np.zeros(64, np.float32)]),
             g=np.asarray(g1), be=np.asarray(be1), s=0.0, mix=mix2),
    ]

    xt_cur = np.ascontiguousarray(xp.T).astype(np.float32)  # [128, NPAD]
    part_cur = np.zeros((128, 16), np.float32)

    import ml_dtypes
    outf = None
    for li, L in enumerate(layers):
        a_s = np.asarray(L["a_src"], np.float32).reshape(-1)  # [128]
        a_d = np.asarray(L["a_dst"], np.float32).reshape(-1)
        asrcT = np.tile(a_s[None, :], (128, 1)).astype(np.float16)
        adstT = np.tile(a_d[None, :], (128, 1)).astype(np.float16)
        prow = (POISON * np.sign(a_s)).astype(np.float32)
        poisr = np.tile(prow[None, :], (128, 1)).astype(np.float16)
        xtb = xt_cur.astype(np.float16)
        in_maps = []
        for c in range(N_CORES):
            # core-rotated feature matrix: own nodes first
            s = c * PER_CORE
            xrot = np.concatenate([xtb[:, s:], xtb[:, :s]], axis=1)
            in_maps.append(dict(
                xt=np.ascontiguousarray(xrot),
                part=part_cur,
                gvec=np.asarray(L["g"], np.float32).reshape(128, 1),
                bevec=np.asarray(L["be"], np.float32).reshape(128, 1),
                srel=np.full((128, 1), L["s"], np.float32),
                wtmat=np.ascontiguousarray(np.asarray(L["W"], np.float32)),
                mmat=np.asarray(L["mix"], np.float16),
                biasv=np.asarray(L["bias"], np.float32).reshape(128, 1),
                asrcT=asrcT,
                adstT=adstT,
                poisr=poisr,
                glo=prep["glo"][c],
                ghi=prep["ghi"][c],
            ))

        tdir = None
        if _TRACE:
            tdir = os.path.join(
                os.environ.get("KERNEL_TRACE_DIR", "/tmp/ktrace"), f"layer{li}")
            os.makedirs(tdir, exist_ok=True)
        res = bass_utils.run_bass_kernel_spmd(
            nc, in_maps, core_ids=list(range(N_CORES)), trace=_TRACE,
            tmpdir=tdir)
        if _TRACE and res.exec_time_ns:
            LAST_EXEC_NS.append(res.exec_time_ns)

        xt_cur = np.concatenate(
            [np.asarray(res.results[c]["outb"], np.float32)
             for c in range(N_CORES)], axis=1)
        part_cur = np.concatenate(
            [np.asarray(res.results[c]["parts"]) for c in range(N_CORES)],
            axis=1).reshape(128, 16)
        # reorder to [sums(8) | sumsq(8)]
        part_cur = np.concatenate(
            [part_cur[:, 0::2], part_cur[:, 1::2]], axis=1)
        if os.environ.get("KERNEL_DEBUG"):
            DBG_LAYERS.append((xt_cur.copy(), part_cur.copy()))
        if li == 2:
            outf = [np.asarray(res.results[c]["outb"]) for c in range(N_CORES)]

    full = np.concatenate(outf, axis=1)  # [128, NPAD]
    out = np.zeros((N, OUT), np.float32)
    out[np.arange(N)] = full[:OUT, :].T[new_id]
    return out
